# revision 13
# baseline (speedup 1.0000x reference)
"""GraphSAGE (mean aggregation) on 8 Trainium2 NeuronCores.

Strategy (v3):
  - Nodes partitioned across 8 cores (6250 real + pad -> 6400/core).
  - Full node-feature table h_all [51200, 128] fp32 replicated in each core's
    DRAM; refreshed with an AllGather after every layer.
  - Edge messages fetched with dma_gather (custom SWDGE gather, 4 queues,
    int16 indices -> table split in two 25600-row halves).
  - Mean aggregation = PE matmuls: per 128-edge block, lhsT = gathered
    messages [128e, 128h] (fp16), rhs = one-hot selection matrix S [128e, 256]
    built on DVE from dst offsets; accumulated in a [128h, 256-node] PSUM
    window; degree counts from the same S with a ones lhsT.
  - Dense SAGE transform per window: zT = Wl^T aggT + Wr^T hT + b, relu on ACT.
  - h' transposed back to node-major via PE and written to cc_in -> AllGather.
"""
import sys

sys.path.insert(0, "/opt/trn_rl_repo")

import numpy as np

import concourse.bass as bass
import concourse.bacc as bacc
import concourse.tile as tile
from concourse import mybir, library_config
from concourse.masks import make_identity

# problem constants (hardcoded per contract)
N, E, IN_DIM, HID, L = 50000, 625000, 300, 128, 4
NC = 8
NPC = N // NC            # 6250 real nodes per core
W_N = 256                # aggregation window width (psum free dim)
NW = 25                  # windows per core
NPAD = W_N * NW          # 6400 padded nodes per core
NTAB = NC * NPAD         # 51200 rows in the replicated table
HALF = NTAB // 2         # 25600 (int16 index limit per gather table slice)
KCH = 3                  # 384 = 3*128 >= IN_DIM contraction chunks
GMAX = 1024              # max indices per dma_gather (descriptor carveout)

_CACHE = {}


def _host_prep(edge_index):
    """Build per-core gather streams, dst-offset blocks and program structure."""
    src = edge_index[0].astype(np.int64)
    dst = edge_index[1].astype(np.int64)
    # padded global ids
    gsrc = (src // NPC) * NPAD + (src % NPC)
    gdst = (dst // NPC) * NPAD + (dst % NPC)

    per_core = []
    counts = np.zeros((NC, 2, NW), dtype=np.int64)
    for m in range(NC):
        sel = (gdst // NPAD) == m
        s_m = gsrc[sel]
        dl = (gdst[sel] - m * NPAD).astype(np.int64)   # 0..6249
        half = (s_m >= HALF).astype(np.int64)
        w = dl // W_N
        # sort by (half, dl) stable
        order = np.lexsort((dl, half))
        s_m, dl, half, w = s_m[order], dl[order], half[order], w[order]
        per_core.append((s_m, dl, half, w))
        for h in range(2):
            cw = np.bincount(w[half == h], minlength=NW)
            counts[m, h, :] = cw

    # uniform block structure across cores
    B = np.zeros((2, NW), dtype=np.int64)
    for h in range(2):
        for w in range(NW):
            B[h, w] = int(np.ceil(counts[:, h, w].max() / 128.0))

    # stream slot layout: per half, concat over windows of B[h,w]*128 slots
    slots_h = [int(B[h].sum() * 128) for h in range(2)]
    nb_h = [int(B[h].sum()) for h in range(2)]

    # per-(h,w) slot offsets
    slot_off = np.zeros((2, NW), dtype=np.int64)
    for h in range(2):
        acc = 0
        for w in range(NW):
            slot_off[h, w] = acc
            acc += B[h, w] * 128

    idx_wrapped = []   # per core: [2][128, slots_h/16] int16
    dof_arr = []       # per core: [2][128, nb_h] fp32
    # gather instruction chunks: per (h, w): list of (slot_off, n)
    chunks = [[[] for _ in range(NW)] for _ in range(2)]
    for h in range(2):
        for w in range(NW):
            nslots = int(B[h, w] * 128)
            off = int(slot_off[h, w])
            while nslots > 0:
                n = min(GMAX, nslots)
                chunks[h][w].append((off, n))
                off += n
                nslots -= n

    for m in range(NC):
        s_m, dl, half, w = per_core[m]
        iw_pair, dof_pair = [], []
        for h in range(2):
            tok = np.zeros(slots_h[h], dtype=np.int16)
            dof = np.full(slots_h[h], -1.0, dtype=np.float32)
            sel = half == h
            s_h, dl_h, w_h = s_m[sel], dl[sel], w[sel]
            for wi in range(NW):
                selw = w_h == wi
                cnt = int(selw.sum())
                if cnt == 0:
                    continue
                o = int(slot_off[h, wi])
                tok[o : o + cnt] = (s_h[selw] - h * HALF).astype(np.int16)
                dof[o : o + cnt] = (dl_h[selw] - wi * W_N).astype(np.float32)
            # wrap idx per gather instruction: j -> [j%16, j//16], replicated x8
            iw = np.zeros((128, slots_h[h] // 16), dtype=np.int16)
            for w0, n in [c for ws in chunks[h] for c in ws]:
                blockw = tok[w0 : w0 + n].reshape(n // 16, 16).T  # [16, n/16]
                iw[:, w0 // 16 : (w0 + n) // 16] = np.tile(blockw, (8, 1))
            iw_pair.append(iw)
            # dstoff partition-major: dof_arr[p, b] = dof[b*128+p]
            dof_pair.append(np.ascontiguousarray(dof.reshape(nb_h[h], 128).T))
        idx_wrapped.append(iw_pair)
        dof_arr.append(dof_pair)

    return {
        "B": B,
        "slots_h": slots_h,
        "nb_h": nb_h,
        "slot_off": slot_off,
        "chunks": chunks,
        "idx_wrapped": idx_wrapped,
        "dof": dof_arr,
    }


def _build_program(struct, timing_reps=1):
    B = struct["B"]
    slots_h = struct["slots_h"]
    nb_h = struct["nb_h"]
    chunks = struct["chunks"]

    nc = bacc.Bacc(
        "TRN2",
        target_bir_lowering=False,
        debug=False,
        num_devices=NC,
        num_swdge_queues=4,
    )
    f32, f16, i16 = mybir.dt.float32, mybir.dt.float16, mybir.dt.int16

    idx_d = [
        nc.dram_tensor(f"idx{h}", [128, max(slots_h[h] // 16, 1)], i16, kind="ExternalInput")
        for h in range(2)
    ]
    dof_d = [
        nc.dram_tensor(f"dof{h}", [128, max(nb_h[h], 1)], f32, kind="ExternalInput")
        for h in range(2)
    ]
    xT_d = nc.dram_tensor("xT", [KCH, 128, NPAD], f16, kind="ExternalInput")
    embW_d = nc.dram_tensor("embW", [KCH, 128, HID], f16, kind="ExternalInput")
    embB_d = nc.dram_tensor("embB", [128, 1], f32, kind="ExternalInput")
    Wl_d = nc.dram_tensor("Wl", [L, 128, HID], f16, kind="ExternalInput")
    Wr_d = nc.dram_tensor("Wr", [L, 128, HID], f16, kind="ExternalInput")
    bl_d = nc.dram_tensor("bl", [L, 128, 1], f32, kind="ExternalInput")
    iota_d = nc.dram_tensor("iota", [128, W_N], f16, kind="ExternalInput")
    ones_d = nc.dram_tensor("ones", [128, 128], f16, kind="ExternalInput")
    out_d = nc.dram_tensor("out", [NPAD, HID], f32, kind="ExternalOutput")

    rg = [list(range(NC))]
    qctr = [0]

    def next_q():
        q = qctr[0] % 4
        qctr[0] += 1
        return q

    import os as _os
    _trace = _os.environ.get("KERNEL_TRACE_SIM") == "1"
    with tile.TileContext(nc, trace_sim=_trace) as tc:
        with (
            tc.tile_pool(name="const", bufs=1) as constp,
            tc.tile_pool(name="big", bufs=1) as bigp,
            tc.tile_pool(name="gt", bufs=3) as gtp,
            tc.tile_pool(name="gt16", bufs=3) as gt16p,
            tc.tile_pool(name="sp", bufs=4) as sp,
            tc.tile_pool(name="ap", bufs=4) as apool,
            tc.tile_pool(name="hp", bufs=4) as hpool,
            tc.tile_pool(name="ivp", bufs=4) as ivp,
            tc.tile_pool(name="pag", bufs=2, space="PSUM") as pag,
            tc.tile_pool(name="pz", bufs=2, space="PSUM") as pz,
            tc.tile_pool(name="pt", bufs=2, space="PSUM") as pt,
            tc.tile_pool(name="pdg", bufs=2, space="PSUM") as pdg,
            tc.tile_pool(name="dram", bufs=1, space="DRAM") as dram,
        ):
            nc.gpsimd.load_library(library_config.mlp)

            # --- resident constants / inputs in SBUF ---
            idx_sb = []
            dof_sb = []
            for h in range(2):
                t = constp.tile([128, max(slots_h[h] // 16, 1)], i16, name=f"idxsb{h}")
                nc.sync.dma_start(out=t[:], in_=idx_d[h][:])
                idx_sb.append(t)
                t2 = constp.tile([128, max(nb_h[h], 1)], f32, name=f"dofsb{h}")
                nc.sync.dma_start(out=t2[:], in_=dof_d[h][:])
                dof_sb.append(t2)
            iota_sb = constp.tile([128, W_N], f16)
            nc.sync.dma_start(out=iota_sb[:], in_=iota_d[:])
            ones_sb = constp.tile([128, 128], f16)
            nc.sync.dma_start(out=ones_sb[:], in_=ones_d[:])
            ident = constp.tile([128, 128], f32)
            make_identity(nc, ident[:])
            ident16 = constp.tile([128, 128], f16)
            nc.vector.tensor_copy(ident16[:], ident[:])
            embW_sb = constp.tile([128, KCH, HID], f16)
            nc.sync.dma_start(out=embW_sb[:], in_=embW_d[:].rearrange("k p h -> p k h"))
            embB_sb = constp.tile([128, 1], f32)
            nc.sync.dma_start(out=embB_sb[:], in_=embB_d[:])
            Wl_sb = constp.tile([128, L, HID], f16)
            nc.sync.dma_start(out=Wl_sb[:], in_=Wl_d[:].rearrange("l p h -> p l h"))
            Wr_sb = constp.tile([128, L, HID], f16)
            nc.sync.dma_start(out=Wr_sb[:], in_=Wr_d[:].rearrange("l p h -> p l h"))
            bl_sb = constp.tile([128, L], f32)
            nc.sync.dma_start(out=bl_sb[:], in_=bl_d[:].rearrange("l p one -> p (l one)"))
            xT_sb = bigp.tile([128, KCH, NPAD], f16)
            nc.sync.dma_start(out=xT_sb[:], in_=xT_d[:].rearrange("k p n -> p k n"))

            inv_sb = bigp.tile([128, NPAD], f32)       # broadcast 1/deg
            hT = [bigp.tile([128, NPAD], f16, name=f"hT{i}") for i in range(2)]

            # DRAM buffers
            n_ag = 1 + timing_reps * (L - 1)
            cc_in = [
                dram.tile([NPAD, HID], f32, name=f"ccin{i}", bufs=1) for i in range(2)
            ]
            h_all = [
                dram.tile([NTAB, HID], f32, name=f"hall{i}", bufs=1, addr_space="Shared")
                for i in range(n_ag)
            ]

            def build_S(h, b_global):
                S = sp.tile([128, W_N], f16, tag="S", name="S")
                nc.vector.tensor_scalar(
                    out=S[:],
                    in0=iota_sb[:],
                    scalar1=dof_sb[h][:, b_global : b_global + 1],
                    scalar2=None,
                    op0=mybir.AluOpType.is_equal,
                )
                return S

            def embedding():
                for w in range(NW):
                    ws = slice(w * W_N, (w + 1) * W_N)
                    pzz = pz.tile([128, W_N], f32, tag="pz", name="pz")
                    for k in range(KCH):
                        nc.tensor.matmul(
                            pzz[:],
                            lhsT=embW_sb[:, k, :],
                            rhs=xT_sb[:, k, ws],
                            start=(k == 0),
                            stop=(k == KCH - 1),
                        )
                    nc.scalar.activation(
                        hT[0][:, ws], pzz[:], mybir.ActivationFunctionType.Relu,
                        bias=embB_sb[:],
                    )
                    writeback(hT[0], w, cc_in[0])

            def writeback(hTbuf, w, dest, dest_f32=None):
                # transpose window back to node-major and DMA to dest rows
                for q in range(2):
                    cs = slice(w * W_N + q * 128, w * W_N + (q + 1) * 128)
                    ptile = pt.tile([128, 128], f16, tag="pt16", name="ptile")
                    nc.tensor.transpose(ptile[:], hTbuf[:, cs], ident16[:])
                    hsb = hpool.tile([128, 128], f32, tag="hsb", name="hsb")
                    nc.vector.tensor_copy(hsb[:], ptile[:])
                    nc.sync.dma_start(out=dest[cs, :], in_=hsb[:])

            def layer(l, h_src, hT_in, hT_out, dest):
                half_ap = [h_src[0:HALF, :], h_src[HALF:NTAB, :]]
                for w in range(NW):
                    pagg = pag.tile([128, W_N], f32, tag="pagg", name="pagg")
                    if l == 0:
                        pdeg = pdg.tile([128, W_N], f32, tag="pdeg", name="pdeg")
                    first = True
                    nblocks = int(B[0][w] + B[1][w])
                    done = 0
                    for h in range(2):
                        boff = int(B[h][:w].sum())
                        lb = 0
                        for (w0, n) in chunks[h][w]:
                            gt = gtp.tile([128, n // 128, 128], f32, tag="gt", name="gt")
                            nc.gpsimd.dma_gather(
                                gt[:],
                                half_ap[h],
                                idx_sb[h][:, w0 // 16 : (w0 + n) // 16],
                                n,
                                n,
                                HID,
                                queue_num=next_q(),
                            )
                            gt16 = gt16p.tile(
                                [128, n // 128, 128], f16, tag="gt16", name="gt16"
                            )
                            nc.any.tensor_copy(gt16[:], gt[:])
                            for j in range(n // 128):
                                S = build_S(h, boff + lb)
                                done += 1
                                nc.tensor.matmul(
                                    pagg[:],
                                    lhsT=gt16[:, j, :],
                                    rhs=S[:],
                                    start=first,
                                    stop=(done == nblocks),
                                )
                                if l == 0:
                                    nc.tensor.matmul(
                                        pdeg[:],
                                        lhsT=ones_sb[:],
                                        rhs=S[:],
                                        start=first,
                                        stop=(done == nblocks),
                                    )
                                first = False
                                lb += 1
                    ws = slice(w * W_N, (w + 1) * W_N)
                    if l == 0:
                        if first:
                            nc.vector.memset(inv_sb[:, ws], 1.0)
                        else:
                            dmax = ivp.tile([128, W_N], f32, tag="dmax", name="dmax")
                            nc.vector.tensor_scalar_max(dmax[:], pdeg[:], 1.0)
                            nc.vector.reciprocal(inv_sb[:, ws], dmax[:])
                    aggT = apool.tile([128, W_N], f16, tag="aggT", name="aggT")
                    if first:
                        nc.vector.memset(aggT[:], 0.0)
                    else:
                        nc.vector.tensor_tensor(
                            out=aggT[:],
                            in0=pagg[:],
                            in1=inv_sb[:, ws],
                            op=mybir.AluOpType.mult,
                        )
                    pzz = pz.tile([128, W_N], f32, tag="pz", name="pz")
                    nc.tensor.matmul(
                        pzz[:], lhsT=Wl_sb[:, l, :], rhs=aggT[:], start=True, stop=False
                    )
                    nc.tensor.matmul(
                        pzz[:], lhsT=Wr_sb[:, l, :], rhs=hT_in[:, ws], start=False,
                        stop=True,
                    )
                    if l < L - 1:
                        nc.scalar.activation(
                            hT_out[:, ws], pzz[:], mybir.ActivationFunctionType.Relu,
                            bias=bl_sb[:, l : l + 1],
                        )
                        writeback(hT_out, w, dest)
                    else:
                        h4 = apool.tile([128, W_N], f16, tag="h4", name="h4")
                        nc.scalar.activation(
                            h4[:], pzz[:], mybir.ActivationFunctionType.Relu,
                            bias=bl_sb[:, l : l + 1],
                        )
                        for q in range(2):
                            cs = slice(w * W_N + q * 128, w * W_N + (q + 1) * 128)
                            ptile = pt.tile([128, 128], f16, tag="pt16", name="ptile")
                            nc.tensor.transpose(
                                ptile[:], h4[:, q * 128 : (q + 1) * 128], ident16[:]
                            )
                            hsb = hpool.tile([128, 128], f32, tag="hsb", name="hsb")
                            nc.vector.tensor_copy(hsb[:], ptile[:])
                            nc.sync.dma_start(out=out_d[cs, :], in_=hsb[:])

            def allgather(src_cc, dst_hall):
                nc.gpsimd.collective_compute(
                    "AllGather",
                    mybir.AluOpType.bypass,
                    replica_groups=rg,
                    ins=[src_cc[:]],
                    outs=[dst_hall[:]],
                )

            embedding()
            allgather(cc_in[0], h_all[0])
            agi = 0
            for rep in range(timing_reps):
                for l in range(L):
                    layer(
                        l,
                        h_all[agi][:],
                        hT[l % 2],
                        hT[(l + 1) % 2],
                        cc_in[(l + 1) % 2],
                    )
                    if l < L - 1:
                        agi += 1
                        allgather(cc_in[(l + 1) % 2], h_all[agi])

    nc.compile()
    return nc


def _prep_inputs(inputs, struct):
    x = np.asarray(inputs["x"], dtype=np.float32)
    emb_W = np.asarray(inputs["emb_W"], dtype=np.float32)
    emb_b = np.asarray(inputs["emb_b"], dtype=np.float32)
    Wl = np.asarray(inputs["Wl"], dtype=np.float32)
    bl = np.asarray(inputs["bl"], dtype=np.float32)
    Wr = np.asarray(inputs["Wr"], dtype=np.float32)

    embW_p = np.zeros((KCH, 128, HID), dtype=np.float16)
    embW_p.reshape(KCH * 128, HID)[:IN_DIM] = emb_W.astype(np.float16)
    embB_p = np.zeros((128, 1), dtype=np.float32)
    embB_p[:, 0] = emb_b
    Wl_p = Wl.astype(np.float16)
    Wr_p = Wr.astype(np.float16)
    bl_p = np.ascontiguousarray(bl[:, :, None].astype(np.float32))

    iota = np.broadcast_to(
        np.arange(W_N, dtype=np.float16)[None, :], (128, W_N)
    ).copy()
    ones = np.ones((128, 128), dtype=np.float16)

    in_maps = []
    for m in range(NC):
        xm = np.zeros((KCH * 128, NPAD), dtype=np.float16)
        xm[:IN_DIM, :NPC] = x[m * NPC : (m + 1) * NPC].T.astype(np.float16)
        im = {
            "idx0": struct["idx_wrapped"][m][0],
            "idx1": struct["idx_wrapped"][m][1],
            "dof0": struct["dof"][m][0],
            "dof1": struct["dof"][m][1],
            "xT": xm.reshape(KCH, 128, NPAD),
            "embW": embW_p,
            "embB": embB_p,
            "Wl": Wl_p,
            "Wr": Wr_p,
            "bl": bl_p,
            "iota": iota,
            "ones": ones,
        }
        in_maps.append(im)
    return in_maps


class BassRunner:
    """Executes a compiled Bass program via PJRT/axon; jit built once."""

    def __init__(self, nc, n_cores):
        import jax
        from jax.sharding import Mesh, PartitionSpec
        from jax.experimental.shard_map import shard_map
        from concourse.bass2jax import (
            _bass_exec_p,
            install_neuronx_cc_hook,
            partition_id_tensor,
        )

        install_neuronx_cc_hook()
        self.jax = jax
        self.nc = nc
        self.n_cores = n_cores
        partition_name = (
            nc.partition_id_tensor.name if nc.partition_id_tensor else None
        )
        in_names, out_names, out_avals, zero_outs = [], [], [], []
        for alloc in nc.m.functions[0].allocations:
            if not isinstance(alloc, mybir.MemoryLocationSet):
                continue
            name = alloc.memorylocations[0].name
            if alloc.kind == "ExternalInput":
                if name != partition_name:
                    in_names.append(name)
            elif alloc.kind == "ExternalOutput":
                shape = tuple(alloc.tensor_shape)
                dtype = mybir.dt.np(alloc.dtype)
                out_names.append(name)
                out_avals.append(jax.core.ShapedArray(shape, dtype))
                zero_outs.append(np.zeros(shape, dtype))
        self.in_names, self.out_names = in_names, out_names
        self.zero_outs, self._out_avals = zero_outs, out_avals
        n_params, n_outs = len(in_names), len(out_avals)
        all_in_names = in_names + out_names
        if partition_name is not None:
            all_in_names = all_in_names + [partition_name]

        def _body(*args):
            operands = list(args)
            if partition_name is not None:
                operands.append(partition_id_tensor())
            return tuple(
                _bass_exec_p.bind(
                    *operands,
                    out_avals=tuple(out_avals),
                    in_names=tuple(all_in_names),
                    out_names=tuple(out_names),
                    lowering_input_output_aliases=(),
                    sim_require_finite=True,
                    sim_require_nnan=True,
                    nc=nc,
                )
            )

        devices = jax.devices()[:n_cores]
        self._mesh = Mesh(np.asarray(devices), ("core",))
        self._pspec = PartitionSpec("core")
        in_specs = (self._pspec,) * (n_params + n_outs)
        out_specs = (self._pspec,) * len(out_names)
        self._fn = jax.jit(
            shard_map(
                _body,
                mesh=self._mesh,
                in_specs=in_specs,
                out_specs=out_specs,
                check_rep=False,
            ),
            keep_unused=True,
        )

    def prepare(self, in_maps):
        n = self.n_cores
        concat_in = [
            np.concatenate(
                [np.asarray(in_maps[c][name]) for c in range(n)], axis=0
            )
            for name in self.in_names
        ]
        concat_zeros = [
            np.zeros((n * z.shape[0], *z.shape[1:]), z.dtype)
            for z in self.zero_outs
        ]
        sharding = self.jax.sharding.NamedSharding(self._mesh, self._pspec)
        self._args = [
            self.jax.device_put(a, sharding) for a in concat_in + concat_zeros
        ]

    def execute(self):
        outs = self._fn(*self._args)
        self.jax.block_until_ready(outs)
        return outs

    def run(self):
        outs = self.execute()
        n = self.n_cores
        return [
            {
                name: np.asarray(outs[i]).reshape(
                    n, *self._out_avals[i].shape
                )[c]
                for i, name in enumerate(self.out_names)
            }
            for c in range(n)
        ]


def _get_runner(edge_index, timing_reps=1):
    key = ("prog", timing_reps, hash(edge_index.tobytes()))
    if key in _CACHE:
        return _CACHE[key]
    struct = _host_prep(edge_index)
    nc = _build_program(struct, timing_reps=timing_reps)
    runner = BassRunner(nc, NC)
    _CACHE[key] = (struct, runner)
    return struct, runner


def kernel(**inputs):
    edge_index = np.asarray(inputs["edge_index"])
    struct, runner = _get_runner(edge_index)
    in_maps = _prep_inputs(inputs, struct)
    runner.prepare(in_maps)
    results = runner.run()
    out = np.empty((N, HID), dtype=np.float32)
    for m in range(NC):
        out[m * NPC : (m + 1) * NPC] = results[m]["out"][:NPC]
    return out


# revision 17
# speedup vs baseline: 43.8230x; 43.8230x over previous
"""GraphSAGE (mean aggregation) on 8 Trainium2 NeuronCores.

Strategy (v3):
  - Nodes partitioned across 8 cores (6250 real + pad -> 6400/core).
  - Full node-feature table h_all [51200, 128] fp32 replicated in each core's
    DRAM; refreshed with an AllGather after every layer.
  - Edge messages fetched with dma_gather (custom SWDGE gather, 4 queues,
    int16 indices -> table split in two 25600-row halves).
  - Mean aggregation = PE matmuls: per 128-edge block, lhsT = gathered
    messages [128e, 128h] (fp16), rhs = one-hot selection matrix S [128e, 256]
    built on DVE from dst offsets; accumulated in a [128h, 256-node] PSUM
    window; degree counts from the same S with a ones lhsT.
  - Dense SAGE transform per window: zT = Wl^T aggT + Wr^T hT + b, relu on ACT.
  - h' transposed back to node-major via PE and written to cc_in -> AllGather.
"""
import sys

sys.path.insert(0, "/opt/trn_rl_repo")

import numpy as np

import concourse.bass as bass
import concourse.bacc as bacc
import concourse.tile as tile
from concourse import mybir, library_config
from concourse.masks import make_identity

# problem constants (hardcoded per contract)
N, E, IN_DIM, HID, L = 50000, 625000, 300, 128, 4
NC = 8
NPC = N // NC            # 6250 real nodes per core
W_N = 256                # aggregation window width (psum free dim)
NW = 25                  # windows per core
NPAD = W_N * NW          # 6400 padded nodes per core
NTAB = NC * NPAD         # 51200 rows in the replicated table
HALF = NTAB // 2         # 25600 (int16 index limit per gather table slice)
KCH = 3                  # 384 = 3*128 >= IN_DIM contraction chunks
GMAX = 1024              # max indices per dma_gather (descriptor carveout)

_CACHE = {}


def _host_prep(edge_index):
    """Build per-core gather streams, dst-offset blocks and program structure."""
    src = edge_index[0].astype(np.int64)
    dst = edge_index[1].astype(np.int64)
    # padded global ids
    gsrc = (src // NPC) * NPAD + (src % NPC)
    gdst = (dst // NPC) * NPAD + (dst % NPC)

    per_core = []
    counts = np.zeros((NC, 2, NW), dtype=np.int64)
    for m in range(NC):
        sel = (gdst // NPAD) == m
        s_m = gsrc[sel]
        dl = (gdst[sel] - m * NPAD).astype(np.int64)   # 0..6249
        half = (s_m >= HALF).astype(np.int64)
        w = dl // W_N
        # sort by (half, dl) stable
        order = np.lexsort((dl, half))
        s_m, dl, half, w = s_m[order], dl[order], half[order], w[order]
        per_core.append((s_m, dl, half, w))
        for h in range(2):
            cw = np.bincount(w[half == h], minlength=NW)
            counts[m, h, :] = cw

    # uniform block structure across cores
    B = np.zeros((2, NW), dtype=np.int64)
    for h in range(2):
        for w in range(NW):
            B[h, w] = int(np.ceil(counts[:, h, w].max() / 128.0))

    # stream slot layout: per half, concat over windows of B[h,w]*128 slots
    slots_h = [int(B[h].sum() * 128) for h in range(2)]
    nb_h = [int(B[h].sum()) for h in range(2)]

    # per-(h,w) slot offsets
    slot_off = np.zeros((2, NW), dtype=np.int64)
    for h in range(2):
        acc = 0
        for w in range(NW):
            slot_off[h, w] = acc
            acc += B[h, w] * 128

    idx_wrapped = []   # per core: [2][128, slots_h/16] int16
    dof_arr = []       # per core: [2][128, nb_h] fp32
    # gather instruction chunks: per (h, w): list of (slot_off, n)
    chunks = [[[] for _ in range(NW)] for _ in range(2)]
    for h in range(2):
        for w in range(NW):
            nslots = int(B[h, w] * 128)
            off = int(slot_off[h, w])
            while nslots > 0:
                n = min(GMAX, nslots)
                chunks[h][w].append((off, n))
                off += n
                nslots -= n

    for m in range(NC):
        s_m, dl, half, w = per_core[m]
        iw_pair, dof_pair = [], []
        for h in range(2):
            tok = np.zeros(slots_h[h], dtype=np.int16)
            dof = np.full(slots_h[h], -1.0, dtype=np.float32)
            sel = half == h
            s_h, dl_h, w_h = s_m[sel], dl[sel], w[sel]
            for wi in range(NW):
                selw = w_h == wi
                cnt = int(selw.sum())
                if cnt == 0:
                    continue
                o = int(slot_off[h, wi])
                tok[o : o + cnt] = (s_h[selw] - h * HALF).astype(np.int16)
                dof[o : o + cnt] = (dl_h[selw] - wi * W_N).astype(np.float32)
            # wrap idx per gather instruction: j -> [j%16, j//16], replicated x8
            iw = np.zeros((128, slots_h[h] // 16), dtype=np.int16)
            for w0, n in [c for ws in chunks[h] for c in ws]:
                blockw = tok[w0 : w0 + n].reshape(n // 16, 16).T  # [16, n/16]
                iw[:, w0 // 16 : (w0 + n) // 16] = np.tile(blockw, (8, 1))
            iw_pair.append(iw)
            # dstoff partition-major: dof_arr[p, b] = dof[b*128+p]
            dof_pair.append(np.ascontiguousarray(dof.reshape(nb_h[h], 128).T))
        idx_wrapped.append(iw_pair)
        dof_arr.append(dof_pair)

    return {
        "B": B,
        "slots_h": slots_h,
        "nb_h": nb_h,
        "slot_off": slot_off,
        "chunks": chunks,
        "idx_wrapped": idx_wrapped,
        "dof": dof_arr,
    }


def _build_program(struct, timing_reps=1):
    B = struct["B"]
    slots_h = struct["slots_h"]
    nb_h = struct["nb_h"]
    chunks = struct["chunks"]

    nc = bacc.Bacc(
        "TRN2",
        target_bir_lowering=False,
        debug=False,
        num_devices=NC,
        num_swdge_queues=4,
    )
    f32, f16, i16 = mybir.dt.float32, mybir.dt.float16, mybir.dt.int16

    idx_d = [
        nc.dram_tensor(f"idx{h}", [128, max(slots_h[h] // 16, 1)], i16, kind="ExternalInput")
        for h in range(2)
    ]
    dof_d = [
        nc.dram_tensor(f"dof{h}", [128, max(nb_h[h], 1)], f32, kind="ExternalInput")
        for h in range(2)
    ]
    xT_d = nc.dram_tensor("xT", [KCH, 128, NPAD], f16, kind="ExternalInput")
    embW_d = nc.dram_tensor("embW", [KCH, 128, HID], f16, kind="ExternalInput")
    embB_d = nc.dram_tensor("embB", [128, 1], f32, kind="ExternalInput")
    Wl_d = nc.dram_tensor("Wl", [L, 128, HID], f16, kind="ExternalInput")
    Wr_d = nc.dram_tensor("Wr", [L, 128, HID], f16, kind="ExternalInput")
    bl_d = nc.dram_tensor("bl", [L, 128, 1], f32, kind="ExternalInput")
    iota_d = nc.dram_tensor("iota", [128, W_N], f16, kind="ExternalInput")
    ones_d = nc.dram_tensor("ones", [128, 128], f16, kind="ExternalInput")
    out_d = nc.dram_tensor("out", [NPAD, HID], f32, kind="ExternalOutput")

    rg = [list(range(NC))]
    qctr = [0]

    def next_q():
        q = qctr[0] % 4
        qctr[0] += 1
        return q

    import os as _os
    _trace = _os.environ.get("KERNEL_TRACE_SIM") == "1"
    _ablate = _os.environ.get("KERNEL_ABLATE") == "1"
    with tile.TileContext(nc, trace_sim=_trace) as tc:
        with (
            tc.tile_pool(name="const", bufs=1) as constp,
            tc.tile_pool(name="big", bufs=1) as bigp,
            tc.tile_pool(name="gt", bufs=8) as gtp,
            tc.tile_pool(name="gt16", bufs=6) as gt16p,
            tc.tile_pool(name="sp", bufs=6) as sp,
            tc.tile_pool(name="ap", bufs=4) as apool,
            tc.tile_pool(name="hp", bufs=4) as hpool,
            tc.tile_pool(name="ivp", bufs=4) as ivp,
            tc.tile_pool(name="pag", bufs=2, space="PSUM") as pag,
            tc.tile_pool(name="pz", bufs=2, space="PSUM") as pz,
            tc.tile_pool(name="pt", bufs=2, space="PSUM") as pt,
            tc.tile_pool(name="pdg", bufs=2, space="PSUM") as pdg,
            tc.tile_pool(name="dram", bufs=1, space="DRAM") as dram,
        ):
            nc.gpsimd.load_library(library_config.mlp)

            # --- resident constants / inputs in SBUF ---
            idx_sb = []
            dof_sb = []
            for h in range(2):
                t = constp.tile([128, max(slots_h[h] // 16, 1)], i16, name=f"idxsb{h}")
                nc.sync.dma_start(out=t[:], in_=idx_d[h][:])
                idx_sb.append(t)
                t2 = constp.tile([128, max(nb_h[h], 1)], f32, name=f"dofsb{h}")
                nc.sync.dma_start(out=t2[:], in_=dof_d[h][:])
                dof_sb.append(t2)
            iota_sb = constp.tile([128, W_N], f16)
            nc.sync.dma_start(out=iota_sb[:], in_=iota_d[:])
            ones_sb = constp.tile([128, 128], f16)
            nc.sync.dma_start(out=ones_sb[:], in_=ones_d[:])
            ident = constp.tile([128, 128], f32)
            make_identity(nc, ident[:])
            ident16 = constp.tile([128, 128], f16)
            nc.vector.tensor_copy(ident16[:], ident[:])
            embW_sb = constp.tile([128, KCH, HID], f16)
            nc.sync.dma_start(out=embW_sb[:], in_=embW_d[:].rearrange("k p h -> p k h"))
            embB_sb = constp.tile([128, 1], f32)
            nc.sync.dma_start(out=embB_sb[:], in_=embB_d[:])
            Wl_sb = constp.tile([128, L, HID], f16)
            nc.sync.dma_start(out=Wl_sb[:], in_=Wl_d[:].rearrange("l p h -> p l h"))
            Wr_sb = constp.tile([128, L, HID], f16)
            nc.sync.dma_start(out=Wr_sb[:], in_=Wr_d[:].rearrange("l p h -> p l h"))
            bl_sb = constp.tile([128, L], f32)
            nc.sync.dma_start(out=bl_sb[:], in_=bl_d[:].rearrange("l p one -> p (l one)"))
            xT_sb = bigp.tile([128, KCH, NPAD], f16)
            nc.sync.dma_start(out=xT_sb[:], in_=xT_d[:].rearrange("k p n -> p k n"))

            inv_sb = bigp.tile([128, NPAD], f32)       # broadcast 1/deg
            hT = [bigp.tile([128, NPAD], f16, name=f"hT{i}") for i in range(2)]

            # DRAM buffers
            n_ag = 1 + timing_reps * (L - 1)
            cc_in = [
                dram.tile([NPAD, HID], f32, name=f"ccin{i}", bufs=1) for i in range(2)
            ]
            h_all = [
                dram.tile([NTAB, HID], f32, name=f"hall{i}", bufs=1, addr_space="Shared")
                for i in range(n_ag)
            ]

            def build_S(h, b_global):
                S = sp.tile([128, W_N], f16, tag="S", name="S")
                nc.vector.tensor_scalar(
                    out=S[:],
                    in0=iota_sb[:],
                    scalar1=dof_sb[h][:, b_global : b_global + 1],
                    scalar2=None,
                    op0=mybir.AluOpType.is_equal,
                )
                return S

            def embedding():
                for w in range(NW):
                    ws = slice(w * W_N, (w + 1) * W_N)
                    pzz = pz.tile([128, W_N], f32, tag="pz", name="pz")
                    for k in range(KCH):
                        nc.tensor.matmul(
                            pzz[:],
                            lhsT=embW_sb[:, k, :],
                            rhs=xT_sb[:, k, ws],
                            start=(k == 0),
                            stop=(k == KCH - 1),
                        )
                    nc.scalar.activation(
                        hT[0][:, ws], pzz[:], mybir.ActivationFunctionType.Relu,
                        bias=embB_sb[:],
                    )
                    writeback(hT[0], w, cc_in[0])

            def writeback(hTbuf, w, dest, dest_f32=None):
                # transpose window back to node-major and DMA to dest rows
                for q in range(2):
                    cs = slice(w * W_N + q * 128, w * W_N + (q + 1) * 128)
                    ptile = pt.tile([128, 128], f16, tag="pt16", name="ptile")
                    nc.tensor.transpose(ptile[:], hTbuf[:, cs], ident16[:])
                    hsb = hpool.tile([128, 128], f32, tag="hsb", name="hsb")
                    nc.vector.tensor_copy(hsb[:], ptile[:])
                    nc.sync.dma_start(out=dest[cs, :], in_=hsb[:])

            def layer(l, h_src, hT_in, hT_out, dest):
                half_ap = [h_src[0:HALF, :], h_src[HALF:NTAB, :]]
                for w in range(NW):
                    pagg = pag.tile([128, W_N], f32, tag="pagg", name="pagg")
                    if l == 0:
                        pdeg = pdg.tile([128, W_N], f32, tag="pdeg", name="pdeg")
                    first = True
                    nblocks = int(B[0][w] + B[1][w])
                    done = 0
                    for h in range(2):
                        boff = int(B[h][:w].sum())
                        lb = 0
                        for (w0, n) in chunks[h][w]:
                            gt = gtp.tile([128, n // 128, 128], f32, tag="gt", name="gt")
                            nc.gpsimd.dma_gather(
                                gt[:],
                                half_ap[h],
                                idx_sb[h][:, w0 // 16 : (w0 + n) // 16],
                                n,
                                n,
                                HID,
                                queue_num=next_q(),
                            )
                            gt16 = gt16p.tile(
                                [128, n // 128, 128], f16, tag="gt16", name="gt16"
                            )
                            if _ablate:
                                lb += n // 128
                                done += n // 128
                                continue
                            nc.any.tensor_copy(gt16[:], gt[:])
                            for j in range(n // 128):
                                S = build_S(h, boff + lb)
                                done += 1
                                nc.tensor.matmul(
                                    pagg[:],
                                    lhsT=gt16[:, j, :],
                                    rhs=S[:],
                                    start=first,
                                    stop=(done == nblocks),
                                )
                                if l == 0:
                                    nc.tensor.matmul(
                                        pdeg[:],
                                        lhsT=ones_sb[:],
                                        rhs=S[:],
                                        start=first,
                                        stop=(done == nblocks),
                                    )
                                first = False
                                lb += 1
                    ws = slice(w * W_N, (w + 1) * W_N)
                    if _ablate:
                        first = True
                    if l == 0:
                        if first:
                            nc.vector.memset(inv_sb[:, ws], 1.0)
                        else:
                            dmax = ivp.tile([128, W_N], f32, tag="dmax", name="dmax")
                            nc.vector.tensor_scalar_max(dmax[:], pdeg[:], 1.0)
                            nc.vector.reciprocal(inv_sb[:, ws], dmax[:])
                    aggT = apool.tile([128, W_N], f16, tag="aggT", name="aggT")
                    if first:
                        nc.vector.memset(aggT[:], 0.0)
                    else:
                        nc.vector.tensor_tensor(
                            out=aggT[:],
                            in0=pagg[:],
                            in1=inv_sb[:, ws],
                            op=mybir.AluOpType.mult,
                        )
                    pzz = pz.tile([128, W_N], f32, tag="pz", name="pz")
                    nc.tensor.matmul(
                        pzz[:], lhsT=Wl_sb[:, l, :], rhs=aggT[:], start=True, stop=False
                    )
                    nc.tensor.matmul(
                        pzz[:], lhsT=Wr_sb[:, l, :], rhs=hT_in[:, ws], start=False,
                        stop=True,
                    )
                    if l < L - 1:
                        nc.vector.tensor_scalar(
                            out=hT_out[:, ws], in0=pzz[:],
                            scalar1=bl_sb[:, l : l + 1], scalar2=0.0,
                            op0=mybir.AluOpType.add, op1=mybir.AluOpType.max,
                        )
                        writeback(hT_out, w, dest)
                    else:
                        h4 = apool.tile([128, W_N], f16, tag="h4", name="h4")
                        nc.vector.tensor_scalar(
                            out=h4[:], in0=pzz[:],
                            scalar1=bl_sb[:, l : l + 1], scalar2=0.0,
                            op0=mybir.AluOpType.add, op1=mybir.AluOpType.max,
                        )
                        for q in range(2):
                            cs = slice(w * W_N + q * 128, w * W_N + (q + 1) * 128)
                            ptile = pt.tile([128, 128], f16, tag="pt16", name="ptile")
                            nc.tensor.transpose(
                                ptile[:], h4[:, q * 128 : (q + 1) * 128], ident16[:]
                            )
                            hsb = hpool.tile([128, 128], f32, tag="hsb", name="hsb")
                            nc.vector.tensor_copy(hsb[:], ptile[:])
                            nc.sync.dma_start(out=out_d[cs, :], in_=hsb[:])

            def allgather(src_cc, dst_hall):
                nc.gpsimd.collective_compute(
                    "AllGather",
                    mybir.AluOpType.bypass,
                    replica_groups=rg,
                    ins=[src_cc[:]],
                    outs=[dst_hall[:]],
                )

            embedding()
            allgather(cc_in[0], h_all[0])
            agi = 0
            for rep in range(timing_reps):
                for l in range(L):
                    layer(
                        l,
                        h_all[agi][:],
                        hT[l % 2],
                        hT[(l + 1) % 2],
                        cc_in[(l + 1) % 2],
                    )
                    if l < L - 1:
                        agi += 1
                        allgather(cc_in[(l + 1) % 2], h_all[agi])

    nc.compile()
    return nc


def _prep_inputs(inputs, struct):
    x = np.asarray(inputs["x"], dtype=np.float32)
    emb_W = np.asarray(inputs["emb_W"], dtype=np.float32)
    emb_b = np.asarray(inputs["emb_b"], dtype=np.float32)
    Wl = np.asarray(inputs["Wl"], dtype=np.float32)
    bl = np.asarray(inputs["bl"], dtype=np.float32)
    Wr = np.asarray(inputs["Wr"], dtype=np.float32)

    embW_p = np.zeros((KCH, 128, HID), dtype=np.float16)
    embW_p.reshape(KCH * 128, HID)[:IN_DIM] = emb_W.astype(np.float16)
    embB_p = np.zeros((128, 1), dtype=np.float32)
    embB_p[:, 0] = emb_b
    Wl_p = Wl.astype(np.float16)
    Wr_p = Wr.astype(np.float16)
    bl_p = np.ascontiguousarray(bl[:, :, None].astype(np.float32))

    iota = np.broadcast_to(
        np.arange(W_N, dtype=np.float16)[None, :], (128, W_N)
    ).copy()
    ones = np.ones((128, 128), dtype=np.float16)

    in_maps = []
    for m in range(NC):
        xm = np.zeros((KCH * 128, NPAD), dtype=np.float16)
        xm[:IN_DIM, :NPC] = x[m * NPC : (m + 1) * NPC].T.astype(np.float16)
        im = {
            "idx0": struct["idx_wrapped"][m][0],
            "idx1": struct["idx_wrapped"][m][1],
            "dof0": struct["dof"][m][0],
            "dof1": struct["dof"][m][1],
            "xT": xm.reshape(KCH, 128, NPAD),
            "embW": embW_p,
            "embB": embB_p,
            "Wl": Wl_p,
            "Wr": Wr_p,
            "bl": bl_p,
            "iota": iota,
            "ones": ones,
        }
        in_maps.append(im)
    return in_maps


class BassRunner:
    """Executes a compiled Bass program via PJRT/axon; jit built once."""

    def __init__(self, nc, n_cores):
        import jax
        from jax.sharding import Mesh, PartitionSpec
        from jax.experimental.shard_map import shard_map
        from concourse.bass2jax import (
            _bass_exec_p,
            install_neuronx_cc_hook,
            partition_id_tensor,
        )

        install_neuronx_cc_hook()
        self.jax = jax
        self.nc = nc
        self.n_cores = n_cores
        partition_name = (
            nc.partition_id_tensor.name if nc.partition_id_tensor else None
        )
        in_names, out_names, out_avals, zero_outs = [], [], [], []
        for alloc in nc.m.functions[0].allocations:
            if not isinstance(alloc, mybir.MemoryLocationSet):
                continue
            name = alloc.memorylocations[0].name
            if alloc.kind == "ExternalInput":
                if name != partition_name:
                    in_names.append(name)
            elif alloc.kind == "ExternalOutput":
                shape = tuple(alloc.tensor_shape)
                dtype = mybir.dt.np(alloc.dtype)
                out_names.append(name)
                out_avals.append(jax.core.ShapedArray(shape, dtype))
                zero_outs.append(np.zeros(shape, dtype))
        self.in_names, self.out_names = in_names, out_names
        self.zero_outs, self._out_avals = zero_outs, out_avals
        n_params, n_outs = len(in_names), len(out_avals)
        all_in_names = in_names + out_names
        if partition_name is not None:
            all_in_names = all_in_names + [partition_name]

        def _body(*args):
            operands = list(args)
            if partition_name is not None:
                operands.append(partition_id_tensor())
            return tuple(
                _bass_exec_p.bind(
                    *operands,
                    out_avals=tuple(out_avals),
                    in_names=tuple(all_in_names),
                    out_names=tuple(out_names),
                    lowering_input_output_aliases=(),
                    sim_require_finite=True,
                    sim_require_nnan=True,
                    nc=nc,
                )
            )

        devices = jax.devices()[:n_cores]
        self._mesh = Mesh(np.asarray(devices), ("core",))
        self._pspec = PartitionSpec("core")
        in_specs = (self._pspec,) * (n_params + n_outs)
        out_specs = (self._pspec,) * len(out_names)
        self._fn = jax.jit(
            shard_map(
                _body,
                mesh=self._mesh,
                in_specs=in_specs,
                out_specs=out_specs,
                check_rep=False,
            ),
            keep_unused=True,
        )

    def prepare(self, in_maps):
        n = self.n_cores
        concat_in = [
            np.concatenate(
                [np.asarray(in_maps[c][name]) for c in range(n)], axis=0
            )
            for name in self.in_names
        ]
        concat_zeros = [
            np.zeros((n * z.shape[0], *z.shape[1:]), z.dtype)
            for z in self.zero_outs
        ]
        sharding = self.jax.sharding.NamedSharding(self._mesh, self._pspec)
        self._args = [
            self.jax.device_put(a, sharding) for a in concat_in + concat_zeros
        ]

    def execute(self):
        outs = self._fn(*self._args)
        self.jax.block_until_ready(outs)
        return outs

    def run(self):
        outs = self.execute()
        n = self.n_cores
        return [
            {
                name: np.asarray(outs[i]).reshape(
                    n, *self._out_avals[i].shape
                )[c]
                for i, name in enumerate(self.out_names)
            }
            for c in range(n)
        ]


def _get_runner(edge_index, timing_reps=1):
    key = ("prog", timing_reps, hash(edge_index.tobytes()))
    if key in _CACHE:
        return _CACHE[key]
    struct = _host_prep(edge_index)
    nc = _build_program(struct, timing_reps=timing_reps)
    runner = BassRunner(nc, NC)
    _CACHE[key] = (struct, runner)
    return struct, runner


def kernel(**inputs):
    edge_index = np.asarray(inputs["edge_index"])
    struct, runner = _get_runner(edge_index)
    in_maps = _prep_inputs(inputs, struct)
    runner.prepare(in_maps)
    results = runner.run()
    out = np.empty((N, HID), dtype=np.float32)
    for m in range(NC):
        out[m * NPC : (m + 1) * NPC] = results[m]["out"][:NPC]
    return out
